# revision 1
# baseline (speedup 1.0000x reference)
"""Multi-head attention (16 heads, d_model=1024, B=2, S=2048) on 8 Trainium2
NeuronCores, tensor-parallel over heads (2 heads per core).

Per-core program (all matmuls bf16 with fp32 PSUM accumulation):
  - q_T/k_T = (W X^T + b) computed in transposed [d, token] layout
  - v in natural [token, d] layout with a ones-column appended (gives the
    softmax denominators for free from the same attn@v matmul)
  - scores_T[j, q] = k_T^T-stationary matmul, exp on ScalarE straight out of
    PSUM (softmax without max-subtraction: scores ~ N(0,1), no overflow risk)
  - unnormalized attn output + denominators accumulate in PSUM; normalization
    applied during eviction via a partition-broadcast reciprocal
  - row block of Wo produces a partial [B*S, 1024] output per core
Host: sum of the 8 partials + (bv @ Wo^T + bo) correction (exact because
softmax rows sum to 1, so the V-bias commutes out of attention).
"""

import numpy as np
import ml_dtypes

import concourse.bass as bass
import concourse.tile as tile
import concourse.bacc as bacc
from concourse import mybir
from concourse import bass_utils

BF16 = ml_dtypes.bfloat16

D_MODEL = 1024
NUM_HEADS = 16
DK = 64
B, S = 2, 2048
BS = B * S
N_CORES = 8
HPC = NUM_HEADS // N_CORES          # heads per core = 2
DPC = HPC * DK                      # head-dim slice per core = 128
P = 128
NF = D_MODEL // P                   # 8 contraction tiles for projections
NIT = BS // P                       # 32 token tiles of 128
SJT = S // P                        # 16 key tiles per batch
FREE = 1024                         # moving free-dim for bf16 matmuls
NQC = BS // FREE                    # 4 projection column chunks
NQT = S // FREE                     # 2 query chunks per batch

f32 = mybir.dt.float32
bf16 = mybir.dt.bfloat16


def _emit(tc, aps, loop=1):
    nc = tc.nc
    xq, xk, xv, wq, wk, wv, wo, bq, bk, out = aps
    QW = 512                       # attention query-chunk width
    NQC2 = S // QW                 # 4 chunks per batch

    import contextlib
    with contextlib.ExitStack() as ctx:
        const = ctx.enter_context(tc.tile_pool(name="const", bufs=1))
        xpool = ctx.enter_context(tc.tile_pool(
            name="xpool", bufs=int(__import__("os").environ.get("XB", "17"))))
        persist = ctx.enter_context(tc.tile_pool(name="persist", bufs=1))
        exp_pool = ctx.enter_context(tc.tile_pool(name="exp", bufs=8))
        attn_pool = ctx.enter_context(tc.tile_pool(name="attnp", bufs=6))
        bc_pool = ctx.enter_context(tc.tile_pool(name="bcast", bufs=3))
        rc_pool = ctx.enter_context(tc.tile_pool(name="recip", bufs=3))
        un_pool = ctx.enter_context(tc.tile_pool(name="unnorm", bufs=4))
        out_pool = ctx.enter_context(tc.tile_pool(name="outp", bufs=2))
        pp_pair = ctx.enter_context(tc.tile_pool(name="pp_pair", bufs=int(__import__("os").environ.get("PPPAIR", "2")), space="PSUM"))
        pp_av = ctx.enter_context(tc.tile_pool(name="pp_av", bufs=int(__import__("os").environ.get("PPAV", "3")), space="PSUM"))
        pp_blk = ctx.enter_context(tc.tile_pool(name="pp_blk", bufs=int(__import__("os").environ.get("PPBLK", "1")), space="PSUM"))

        # ---- constants ----
        wq_sb = const.tile([P, NF, P], bf16)
        wk_sb = const.tile([P, NF, P], bf16)
        wv_sb = const.tile([P, NF, P], bf16)
        wo_sb = const.tile([P, D_MODEL], bf16)
        for w_sb, w_ap in ((wq_sb, wq), (wk_sb, wk), (wv_sb, wv)):
            nc.sync.dma_start(w_sb[:], w_ap.rearrange("(n p) m -> p n m", p=P))
        nc.sync.dma_start(wo_sb[:], wo[:])
        bq_sb = const.tile([P, 1], f32)
        bk_sb = const.tile([P, 1], f32)
        nc.sync.dma_start(bq_sb[:], bq[:])
        nc.sync.dma_start(bk_sb[:], bk[:])

        q_sb = persist.tile([P, BS], bf16)
        k_sb = persist.tile([P, BS], bf16)
        v_sb = persist.tile([P, NIT, 2 * (DK + 1)], bf16)

        import os as _osc
        cet = None
        if _osc.environ.get("CONSTET") == "1":
            cet = const.tile([P, HPC, 512], bf16)
            nc.vector.memset(cet[:], 0.5)

        # ones columns of v_aug (softmax denominator rows)
        nc.vector.memset(v_sb[:, :, DK : DK + 1], 1.0)
        nc.vector.memset(v_sb[:, :, 2 * DK + 1 : 2 * DK + 2], 1.0)

        import os as _osf
        NOX = _osf.environ.get("NOX") == "1"
        NOPROJ = _osf.environ.get("NOPROJ") == "1"
        NOWO = _osf.environ.get("NOWO") == "1"
        NOCHAIN = _osf.environ.get("NOCHAIN") == "1"
        if NOPROJ:
            nc.vector.memset(q_sb[:], 0.1)
            nc.vector.memset(k_sb[:], 0.1)
            nc.vector.memset(v_sb[:], 0.1)
        if NOCHAIN:
            pass

        import contextlib as _ctl
        loop_cm = tc.For_i(0, loop, 1) if loop > 1 else _ctl.nullcontext()
        with loop_cm:
            xt = {}

            def load_x(b):
                QWC = S // QW
                srcq = xq[:, b * S : (b + 1) * S].rearrange("(n p) m -> n p m", p=P)
                for name, x_ap in (("k", xk), ("v", xv)):
                    src = x_ap[:, b * S : (b + 1) * S].rearrange("(n p) m -> n p m", p=P)
                    tiles = []
                    for f in range(NF):
                        t = xpool.tile([P, S], bf16, tag="x")
                        nc.sync.dma_start(t[:], src[f])
                        tiles.append(t)
                    xt[(name, b)] = tiles
                    if name == "k":
                        # first q chunk right after k, before the bulk of v
                        qtiles = [[None] * QWC for _ in range(NF)]
                        xt[("q", b)] = qtiles
                        for f in range(NF):
                            t = xpool.tile([P, QW], bf16, tag="xq",
                                           bufs=int(__import__("os").environ.get("XQB", "34")),
                                           name=f"xq{f}c0")
                            nc.sync.dma_start(t[:], srcq[f, :, 0:QW])
                            qtiles[f][0] = t
                for c in range(1, QWC):
                    for f in range(NF):
                        t = xpool.tile([P, QW], bf16, tag="xq",
                                       bufs=int(__import__("os").environ.get("XQB", "34")),
                                       name=f"xq{f}c{c}")
                        nc.sync.dma_start(t[:], srcq[f, :, c * QW : (c + 1) * QW])
                        xt[("q", b)][f][c] = t

            def emit_qk(kind, b, c):
                w_sb, b_sb, dest = ((wq_sb, bq_sb, q_sb) if kind == "q"
                                    else (wk_sb, bk_sb, k_sb))
                ps = pp_blk.tile([P, QW], f32, tag="blk")
                cs = slice(c * QW, (c + 1) * QW)  # local within batch
                for f in range(NF):
                    rhs = (xt[(kind, b)][f][c][:]
                           if kind == "q" else xt[(kind, b)][f][:, cs])
                    nc.tensor.matmul(ps[:], w_sb[:, f, :], rhs,
                                     start=(f == 0), stop=(f == NF - 1))
                nc.vector.tensor_scalar_add(
                    dest[:, b * S + c * QW : b * S + (c + 1) * QW], ps[:], b_sb[:])

            def emit_v(b, it2):
                ps = pp_blk.tile([P, QW], f32, tag="blk")
                isl = slice(it2 * P, (it2 + 1) * P)
                for f in range(NF):
                    nc.tensor.matmul(ps[:, 0:P], xt[("v", b)][f][:, isl], wv_sb[:, f, :],
                                     start=(f == 0), stop=(f == NF - 1))
                dst = v_sb[:, b * SJT + it2, 0:DK]
                dst = bass.AP(dst.tensor, dst.offset, [dst.ap[0], [DK + 1, 2], [1, DK]])
                nc.vector.tensor_copy(dst, ps[:, 0:P].rearrange("p (a b) -> p a b", a=2))

            def emit_wo(attn_c, b, qc, i2):
                po = pp_blk.tile([P, QW], f32, tag="blk")
                nc.tensor.matmul(po[:], attn_c[:, i2 * P : (i2 + 1) * P],
                                 wo_sb[:, 0:QW], start=True, stop=True)
                po2 = pp_blk.tile([P, QW], f32, tag="blk")
                nc.tensor.matmul(po2[:], attn_c[:, i2 * P : (i2 + 1) * P],
                                 wo_sb[:, QW:], start=True, stop=True)
                import os as _oso
                ot = out_pool.tile([P, D_MODEL],
                                   bf16 if _oso.environ.get("OUTBF", "1") == "1" else f32)
                import os as _osw
                if _osw.environ.get("WOACT", "0") == "1":
                    nc.scalar.copy(ot[:, 0:QW], po[:])
                    nc.scalar.copy(ot[:, QW:], po2[:])
                else:
                    nc.vector.tensor_copy(ot[:, 0:QW], po[:])
                    nc.vector.tensor_copy(ot[:, QW:], po2[:])
                row0 = b * S + qc * QW + i2 * P
                nc.sync.dma_start(out[row0 : row0 + P, :], ot[:])

            pending = []
            done = set()

            def emit_block(blk):
                key = blk[:1] + tuple(x for x in blk[1:] if not hasattr(x, "tensor"))
                if blk[0] == "qk_q":
                    emit_qk("q", blk[1], blk[2])
                elif blk[0] == "qk_k":
                    emit_qk("k", blk[1], blk[2])
                elif blk[0] == "v":
                    emit_v(blk[1], blk[2])
                else:
                    emit_wo(blk[1], blk[2], blk[3], blk[4])
                done.add(key)

            def force(key):
                if NOPROJ or key in done:
                    return
                for i, blk in enumerate(pending):
                    bkey = blk[:1] + tuple(x for x in blk[1:] if not hasattr(x, "tensor"))
                    if bkey == key:
                        pending.pop(i)
                        emit_block(blk)
                        return
                raise KeyError(key)

            def drain(n):
                for _ in range(min(n, len(pending))):
                    emit_block(pending.pop(0))

            for b in range(B):
                if not NOX:
                    load_x(b)
                if not NOPROJ:
                    for c in range(NQC2):
                        pending.append(("qk_k", b, c))
                    pending.append(("qk_q", b, 0))
                    for it2 in range(SJT):
                        pending.append(("v", b, it2))
                    for c in range(1, NQC2):
                        pending.append(("qk_q", b, c))

            import os as _os2
            if _os2.environ.get("PROJONLY") == "1":
                drain(len(pending))
                return
            for b in range(B):
                # prologue for this batch: k fully, first q chunk
                for c in range(NQC2):
                    force(("qk_k", b, c))

                for qc in range(NQC2):
                    force(("qk_q", b, qc))
                    qss = slice(b * S + qc * QW, b * S + (qc + 1) * QW)
                    attn_c = attn_pool.tile([P, QW], bf16, tag="attn")
                    pav = [pp_av.tile([DK + 1, QW], f32, tag="av", name=f"pav{h}")
                           for h in range(HPC)]
                    SKEW = int(__import__("os").environ.get("SKEW", "2"))
                    ets = {}
                    for jt in range(SJT + SKEW):
                        if jt < SJT:
                            for la in range(SKEW + 1):
                                if jt + la < SJT:
                                    force(("v", b, jt + la))
                            jsl = slice(b * S + jt * P, b * S + (jt + 1) * P)
                            pair = pp_pair.tile([P, HPC, QW], f32, tag="pair")
                            for h in range(HPC):
                                nc.tensor.matmul(
                                    pair[:, h, :], k_sb[h * DK : (h + 1) * DK, jsl],
                                    q_sb[h * DK : (h + 1) * DK, qss],
                                    start=True, stop=True,
                                )
                            import os as _os3
                            if _os3.environ.get("CONSTET") == "1":
                                ets[jt] = cet
                            else:
                                et = exp_pool.tile([P, HPC, QW], bf16)
                                if _os3.environ.get("EXPSPLIT") == "1":
                                    for h in range(HPC):
                                        nc.scalar.activation(
                                            et[:, h, :], pair[:, h, :],
                                            mybir.ActivationFunctionType.Exp, scale=0.125,
                                        )
                                else:
                                    nc.scalar.activation(
                                        et[:], pair[:],
                                        mybir.ActivationFunctionType.Exp, scale=0.125,
                                    )
                                ets[jt] = et
                        ja = jt - SKEW
                        if ja >= 0:
                            et = ets.pop(ja)
                            for h in range(HPC):
                                nc.tensor.matmul(
                                    pav[h][:],
                                    v_sb[:, b * SJT + ja, h * (DK + 1) : (h + 1) * (DK + 1)],
                                    et[:, h, :],
                                    start=(ja == 0), stop=(ja == SJT - 1),
                                )
                        drain(1)
                        if jt == 9:
                            nb, nqc = (b, qc + 1) if qc + 1 < NQC2 else (b + 1, 0)
                            if nb < B:
                                force(("qk_q", nb, nqc))
                    for h in range(HPC):
                        if NOCHAIN:
                            continue
                        import os as _os
                        if _os.environ.get("LNCHAIN", "1") == "1":
                            # reciprocal of the softmax sums via exp(-ln(s)) on
                            # ScalarE (same activation-table set as the score
                            # exp), keeping the slow DVE InstReciprocal off the
                            # critical path entirely.
                            lnr = rc_pool.tile([1, QW], f32)
                            nc.scalar.activation(
                                lnr[:], pav[h][DK : DK + 1, :],
                                mybir.ActivationFunctionType.Ln)
                            rec = rc_pool.tile([1, QW], f32, name="rec")
                            nc.scalar.activation(
                                rec[:], lnr[:],
                                mybir.ActivationFunctionType.Exp, scale=-1.0)
                            bc = bc_pool.tile([DK, QW], f32)
                            nc.gpsimd.partition_broadcast(bc[:], rec[:])
                            nc.vector.tensor_mul(
                                attn_c[h * DK : (h + 1) * DK, :], pav[h][0:DK, :], bc[:])
                            continue
                        if _os.environ.get("DIVCHAIN", "0") == "1":
                            # evict pav to SBUF first: frees the PSUM slot after
                            # one op; the normalization chain then runs on SBUF
                            # with no PSUM slot held.
                            un = un_pool.tile([DK + 1, QW], f32)
                            nc.scalar.copy(un[:], pav[h][:])
                            rc = rc_pool.tile([1, QW], f32)
                            nc.vector.reciprocal(rc[:], un[DK : DK + 1, :])
                            bc = bc_pool.tile([DK, QW], f32)
                            nc.gpsimd.partition_broadcast(bc[:], rc[:])
                            nc.vector.tensor_mul(
                                attn_c[h * DK : (h + 1) * DK, :], un[0:DK, :], bc[:])
                            continue
                        rc = rc_pool.tile([1, QW], f32)
                        nc.vector.reciprocal(rc[:], pav[h][DK : DK + 1, :])
                        bc = bc_pool.tile([DK, QW], f32)
                        if _os.environ.get("NOBCAST") == "1":
                            nc.vector.memset(bc[:], 1.0)   # timing-only variant
                        else:
                            nc.gpsimd.partition_broadcast(bc[:], rc[:])
                        nc.vector.tensor_mul(
                            attn_c[h * DK : (h + 1) * DK, :], pav[h][0:DK, :], bc[:])
                    if not NOWO:
                        for i2 in range(QW // P):
                            pending.append(("wo", attn_c, b, qc, i2))

            drain(len(pending))


def _build(loop=1):
    nc = bacc.Bacc("TRN2", target_bir_lowering=False, debug=False,
                   num_devices=N_CORES)
    xq = nc.dram_tensor("xq_t", [D_MODEL, BS], bf16, kind="ExternalInput").ap()
    xk = nc.dram_tensor("xk_t", [D_MODEL, BS], bf16, kind="ExternalInput").ap()
    xv = nc.dram_tensor("xv_t", [D_MODEL, BS], bf16, kind="ExternalInput").ap()
    wq = nc.dram_tensor("wq_t", [D_MODEL, DPC], bf16, kind="ExternalInput").ap()
    wk = nc.dram_tensor("wk_t", [D_MODEL, DPC], bf16, kind="ExternalInput").ap()
    wv = nc.dram_tensor("wv_t", [D_MODEL, DPC], bf16, kind="ExternalInput").ap()
    wo = nc.dram_tensor("wo_t", [DPC, D_MODEL], bf16, kind="ExternalInput").ap()
    bq = nc.dram_tensor("bq", [DPC, 1], f32, kind="ExternalInput").ap()
    bk = nc.dram_tensor("bk", [DPC, 1], f32, kind="ExternalInput").ap()
    ob16 = __import__("os").environ.get("OUTBF", "1") == "1"
    out = nc.dram_tensor("out_p", [BS, D_MODEL], bf16 if ob16 else f32,
                         kind="ExternalOutput").ap()

    with tile.TileContext(nc) as tc:
        _emit(tc, (xq, xk, xv, wq, wk, wv, wo, bq, bk, out), loop=loop)
    nc.compile()
    return nc


_cache = {}


def _get_nc(loop=1):
    import os
    key = (loop, os.environ.get("XB", "17"), os.environ.get("XQB", "34"), os.environ.get("OUTBF", "1"), os.environ.get("SKEW", "2"), os.environ.get("PPPAIR", "2"),
           os.environ.get("PPAV", "3"), os.environ.get("PPBLK", "1"), os.environ.get("NOEXP", ""), os.environ.get("EXPSPLIT", ""), os.environ.get("CONSTET", ""), os.environ.get("PROJONLY", ""), os.environ.get("NOX", ""), os.environ.get("NOPROJ", ""), os.environ.get("NOWO", ""), os.environ.get("NOCHAIN", ""), os.environ.get("DIVCHAIN", "0"), os.environ.get("LNCHAIN", "1"), os.environ.get("WOACT", "0"), os.environ.get("LNCHAIN", "1"))
    if key not in _cache:
        _cache[key] = _build(loop)
    return _cache[key]


def _make_in_maps(Q, K, V, Wq, bq, Wk, bk, Wv, bv, Wo, bo):
    xq_t = np.ascontiguousarray(np.asarray(Q, np.float32).reshape(BS, D_MODEL).T).astype(BF16)
    xk_t = np.ascontiguousarray(np.asarray(K, np.float32).reshape(BS, D_MODEL).T).astype(BF16)
    xv_t = np.ascontiguousarray(np.asarray(V, np.float32).reshape(BS, D_MODEL).T).astype(BF16)
    in_maps = []
    for c in range(N_CORES):
        sl = slice(c * DPC, (c + 1) * DPC)
        in_maps.append({
            "xq_t": xq_t, "xk_t": xk_t, "xv_t": xv_t,
            "wq_t": np.ascontiguousarray(np.asarray(Wq)[sl].T).astype(BF16),
            "wk_t": np.ascontiguousarray(np.asarray(Wk)[sl].T).astype(BF16),
            "wv_t": np.ascontiguousarray(np.asarray(Wv)[sl].T).astype(BF16),
            "wo_t": np.ascontiguousarray(np.asarray(Wo)[:, sl].T).astype(BF16),
            "bq": np.asarray(bq, np.float32)[sl].reshape(DPC, 1).copy(),
            "bk": np.asarray(bk, np.float32)[sl].reshape(DPC, 1).copy(),
        })
    return in_maps


def kernel(Q, K, V, Wq, bq, Wk, bk, Wv, bv, Wo, bo):
    nc = _get_nc()
    in_maps = _make_in_maps(Q, K, V, Wq, bq, Wk, bk, Wv, bv, Wo, bo)
    res = bass_utils.run_bass_kernel_spmd(nc, in_maps, core_ids=list(range(N_CORES)))
    acc = np.zeros((BS, D_MODEL), np.float32)
    for c in range(N_CORES):
        acc += np.asarray(res.results[c]["out_p"], np.float32)
    corr = (np.asarray(bv, np.float64) @ np.asarray(Wo, np.float64).T
            + np.asarray(bo, np.float64)).astype(np.float32)
    return (acc + corr[None, :]).reshape(B, S, D_MODEL).astype(np.float32)

